# revision 1
# baseline (speedup 1.0000x reference)
"""CLIP cross-attention kernel for 8 TRN2 NeuronCores (v2).

Math (per batch b, head h):
  Q = (T @ Wq + bq) * scale           T = text_states[:, b, :]   (128, 1024)
  K = X @ Wk + bk                     X = hidden_states[b]       (4096, 1024)
  V = X @ Wv + bv
  S = Q_h @ K_h^T                     (128, 4096)
  E = exp(S); d = rowsum(E)
  out_h = E^T @ (E @ V_h) / d^2       (4096, 64)
  final = concat_h(out_h) @ Wo + bo

Sharding: batch across 8 cores (2 batches each), weights replicated.

Design vs the original baseline (1343us -> 1170us measured):
 - K^T and V stay RESIDENT in SBUF (no DRAM round trip, no 348K-descriptor
   DMA storm). X is streamed in 512-column chunks during the projections.
 - K^T and Q^T are stored as fp8e4 (scaled by 8): S matmuls run fp8 and
   kt shrinks to 32KB/partition. Measured end-to-end rel err 7.4e-3.
 - E^T is produced by the DMA xbar transpose engine (dma_start_transpose,
   4x 1024-col calls per head on the sync queue), replacing the S^T matmul
   pass + second exp: saves ~135us of PE and ~170us of ACT per core.
 - Attention is software-pipelined per head with emission order
   u(h-1)[0:16] | S(h)[g0g1] | u(h-1)[16:32] | S(h)[g2g3] | chain(h-1) |
   out(h-1), out-psum tiles alternating two PSUM pools so the PE is not
   paced by the DVE evacuation casts; ot writes back as ONE DMA per head
   (8 small DMAs per head saturated the sync queue and starved the
   transposes -> u-matmul stalls).
 - V columns 512:1023 (only needed by heads 8-15) are computed inside the
   attention loop of heads 0-7 as dense PE filler (keeps HAM clock warm).
 - 1/d^2 is applied via a per-partition scale AP on the ACT evacuation of
   the u accumulator; biases ride along activation evacuations for free.
 - All pools hoisted to top level; phases of consecutive batches overlap.
 - NOTE: dispatching DMAs from the ACT hwdge queue (nc.scalar.dma_start)
   hard-crashed the device (NRT_EXEC_UNIT_UNRECOVERABLE) - keep all DMAs
   on nc.sync.
"""
import sys
import numpy as np

sys.path.insert(0, '/opt/trn_rl_repo')

import concourse.bass as bass          # noqa: E402
import concourse.tile as tile          # noqa: E402
from concourse import bacc, mybir      # noqa: E402
from concourse import bass_utils       # noqa: E402
from contextlib import ExitStack       # noqa: E402

DT = mybir.dt.float32
BF = mybir.dt.bfloat16
F8 = mybir.dt.float8e4
AF = mybir.ActivationFunctionType

B, LT, LV, D, H = 16, 128, 4096, 1024, 16
HD = D // H          # 64
NB = 2               # batches per core
N_CORES = 8
SCALE = HD ** -0.5
KD = D // 128        # 8
LVT = LV // 128      # 32
NCH = LV // 512      # 8
QK8 = 8.0            # fp8 storage scale for q/k


def build_program(nb=NB, with_bv=False):
    nc = bacc.Bacc("TRN2", target_bir_lowering=False, debug=False)

    xt_d = nc.dram_tensor("xt", [nb, D, LV], BF, kind="ExternalInput")
    tt_d = nc.dram_tensor("tt", [nb, D, LT], BF, kind="ExternalInput")
    w_d = {nm: nc.dram_tensor(nm, [D, D], BF, kind="ExternalInput")
           for nm in ("wq", "wk", "wv", "wo")}
    b_d = {nm: nc.dram_tensor(nm, [D], DT, kind="ExternalInput")
           for nm in ("bqs8", "bk8", "bo")}
    if with_bv:
        b_d["bv"] = nc.dram_tensor("bv", [D], DT, kind="ExternalInput")
    out_d = nc.dram_tensor("out", [nb, D, LV], DT, kind="ExternalOutput")
    ot_d = nc.dram_tensor("ot_scratch", [nb, D, LV], BF)

    with tile.TileContext(nc) as tc, ExitStack() as top:
        ep = top.enter_context
        wpool = ep(tc.tile_pool(name="wp", bufs=2))
        biasp = ep(tc.tile_pool(name="biasp", bufs=1))
        xtp = ep(tc.tile_pool(name="xtp", bufs=2))
        ktp = ep(tc.tile_pool(name="ktp", bufs=1))
        vp = ep(tc.tile_pool(name="vp", bufs=1))
        ttp = ep(tc.tile_pool(name="ttp", bufs=1))
        qtp = ep(tc.tile_pool(name="qtp", bufs=1))
        enp = ep(tc.tile_pool(name="enp", bufs=2))
        etp = ep(tc.tile_pool(name="etp", bufs=3))
        smp = ep(tc.tile_pool(name="smp", bufs=2))
        otsg = ep(tc.tile_pool(name="otsg", bufs=2))
        fosg = ep(tc.tile_pool(name="fosg", bufs=2))
        psA = ep(tc.tile_pool(name="psA", bufs=2, space="PSUM"))
        psB = ep(tc.tile_pool(name="psB", bufs=2, space="PSUM"))
        psC = ep(tc.tile_pool(name="psC", bufs=2, space="PSUM"))

        def load_weight(nm):
            t = wpool.tile([128, KD, D], BF, name=f"w_{nm}", tag="w")
            src = w_d[nm].ap().rearrange("(k p) n -> p k n", p=128)
            for k in range(KD):
                nc.sync.dma_start(t[:, k, :], src[:, k, :])
            return t

        bias_sb = {}
        for nm in b_d:
            t = biasp.tile([128, KD], DT, name=f"b_{nm}", tag=f"b_{nm}")
            nc.sync.dma_start(t[:], b_d[nm].ap().rearrange("(k p) -> p k", p=128))
            bias_sb[nm] = t

        bv_bcast = None
        if with_bv:
            bv_row = biasp.tile([1, D], DT, tag="bv_row")
            nc.sync.dma_start(bv_row[:], b_d["bv"].ap().unsqueeze(0))
            ones_row = biasp.tile([1, 128], DT, tag="ones_row")
            nc.vector.memset(ones_row[:], 1.0)
            bv_bcast = biasp.tile([128, D], DT, tag="bv_bcast")
            for g in range(2):
                pb = psB.tile([128, 512], DT, name="bv_ps", tag="B")
                nc.tensor.matmul(pb[:], ones_row[:],
                                 bv_row[:, 512 * g:512 * (g + 1)])
                nc.vector.tensor_copy(bv_bcast[:, 512 * g:512 * (g + 1)], pb[:])

        for b in range(nb):
            # ---------- P1: K^T (fp8, resident) + V (bf16, resident) ----------
            p1_scope = nc.named_scope(f"p1_b{b}"); p1_scope.__enter__()
            wk_sb = load_weight("wk")
            wv_sb = load_weight("wv")
            kt_sb = ktp.tile([128, KD, LV], F8, name="kt", tag="kt")
            v_sb = vp.tile([128, LVT, D], BF, name="v", tag="v")
            xsrc = xt_d[b].rearrange("(k p) n -> p k n", p=128)

            for cp in range(LV // 1024):
                xts = []
                for half in range(2):
                    c0 = 1024 * cp + 512 * half
                    xt_t = xtp.tile([128, KD, 512], BF, name="xt_c", tag="xt")
                    nc.sync.dma_start(xt_t[:], xsrc[:, :, c0:c0 + 512])
                    xts.append(xt_t)
                # K^T: out rows m-block, cols = this 1024-chunk
                for m in range(KD):
                    ps = psA.tile([128, 1024], DT, name="k_ps", tag="A")
                    for k in range(KD):
                        lw = wk_sb[:, k, 128 * m:128 * (m + 1)]
                        for half in range(2):
                            nc.tensor.matmul(ps[:, 512 * half:512 * (half + 1)],
                                             lw, xts[half][:, k, :],
                                             start=(k == 0), stop=(k == KD - 1))
                    nc.scalar.activation(
                        kt_sb[:, m, 1024 * cp:1024 * (cp + 1)], ps[:],
                        AF.Identity, bias=bias_sb["bk8"][:, m:m + 1], scale=QK8)
                # V cols 0:512 (heads 0-7) here; cols 512:1024 are computed
                # inside the attention loop as PE filler (emit_vg1).
                for half in range(2):
                    for tl in range(4):
                        t_abs = 8 * cp + 4 * half + tl
                        ps = psB.tile([128, 512], DT, name="v_ps", tag="B")
                        for k in range(KD):
                            nc.tensor.matmul(
                                ps[:],
                                xts[half][:, k, 128 * tl:128 * (tl + 1)],
                                wv_sb[:, k, 0:512],
                                start=(k == 0), stop=(k == KD - 1))
                        dst = v_sb[:, t_abs, 0:512]
                        if with_bv:
                            nc.vector.tensor_add(
                                dst, ps[:], bv_bcast[:, 0:512])
                        else:
                            nc.vector.tensor_copy(dst, ps[:])

            p1_scope.__exit__(None, None, None)
            # ---------- P2: Q^T (fp8) ----------
            p2_scope = nc.named_scope(f"p2_b{b}"); p2_scope.__enter__()
            tt_sb = ttp.tile([128, KD, LT], BF, name="tt", tag="tt")
            nc.sync.dma_start(tt_sb[:], tt_d[b].rearrange("(k p) t -> p k t", p=128))
            wq_sb = load_weight("wq")
            qt_sb = qtp.tile([128, KD, LT], F8, name="qt", tag="qt")
            for m in range(KD):
                ps = psC.tile([128, LT], DT, name="q_ps", tag="C")
                for k in range(KD):
                    nc.tensor.matmul(ps[:], wq_sb[:, k, 128 * m:128 * (m + 1)],
                                     tt_sb[:, k, :],
                                     start=(k == 0), stop=(k == KD - 1))
                nc.scalar.activation(qt_sb[:, m, :], ps[:], AF.Identity,
                                     bias=bias_sb["bqs8"][:, m:m + 1],
                                     scale=SCALE * QK8)

            wo_sb = load_weight("wo")   # prefetch during attention
            p2_scope.__exit__(None, None, None)
            p3_scope = nc.named_scope(f"p3_b{b}"); p3_scope.__enter__()

            # ---------- P3: attention, software-pipelined over heads ----------
            # Emission order per head h keeps the PE fed while the ACT exp /
            # DVE normalization chains for neighbouring heads complete:
            #   u(h-1)[0:16] | S(h)[g0,g1] | u(h-1)[16:32] | S(h)[g2,g3] |
            #   chain(h-1) | out(h-1) | casts+DMA(h-1)
            live = {}

            def emit_S(h, glo, ghi):
                p, hb = h // 2, 64 * (h % 2)
                qth = qt_sb[hb:hb + 64, p, :]
                if glo == 0:
                    live[h] = {
                        "en": enp.tile([128, LV], BF, name="en", tag="en"),
                        "et": etp.tile([128, LVT, 128], BF, name="et", tag="et"),
                        "dparts": smp.tile([128, 4], DT, name="dparts",
                                           tag="dparts"),
                    }
                st = live[h]
                for g in range(glo, ghi):
                    ps = psA.tile([128, 1024], DT, name="s_ps", tag="A")
                    for half in range(2):
                        c0 = 1024 * g + 512 * half
                        nc.tensor.matmul(ps[:, 512 * half:512 * (half + 1)],
                                         qth, kt_sb[hb:hb + 64, p, c0:c0 + 512])
                    nc.scalar.activation(
                        st["en"][:, 1024 * g:1024 * (g + 1)], ps[:], AF.Exp,
                        scale=1.0 / (QK8 * QK8),
                        accum_out=st["dparts"][:, g:g + 1])
                    nc.sync.dma_start_transpose(
                        st["et"][:, 8 * g:8 * (g + 1), :],
                        st["en"][:, 1024 * g:1024 * (g + 1)])

            def emit_u(h, tlo, thi):
                st = live[h]
                if tlo == 0:
                    st["ub"] = psC.tile([128, HD], DT, name="u_ps", tag="C")
                for t in range(tlo, thi):
                    nc.tensor.matmul(st["ub"][:], st["et"][:, t, :],
                                     v_sb[:, t, HD * h:HD * (h + 1)],
                                     start=(t == 0), stop=(t == LVT - 1))

            def emit_chain(h):
                st = live[h]
                dsum = smp.tile([128, 1], DT, name="dsum", tag="dsum")
                nc.vector.reduce_sum(dsum[:], st["dparts"][:],
                                     axis=mybir.AxisListType.X)
                rd = smp.tile([128, 1], DT, name="rd", tag="rd")
                nc.vector.reciprocal(rd[:], dsum[:])
                rr = smp.tile([128, 1], DT, name="rr", tag="rr")
                nc.vector.tensor_mul(rr[:], rd[:], rd[:])
                up = smp.tile([128, HD], BF, name="up", tag="up")
                # per-partition scale AP applies 1/d^2 during PSUM evacuation
                nc.scalar.activation(up[:], st["ub"][:], AF.Identity,
                                     scale=rr[:])
                st["up"] = up

            def emit_vg1(h, tlo, thi):
                # V columns 512:1024 (heads 8-15) for vis-tiles 4h+tlo..4h+thi,
                # emitted inside the attention loop as dense PE filler.
                xtb = vg1_chunks[h]
                for tl in range(tlo, thi):
                    t_abs = 4 * h + tl
                    ps = psB.tile([128, 512], DT, name="v_ps1", tag="B")
                    for k in range(KD):
                        nc.tensor.matmul(
                            ps[:], xtb[:, k, 128 * tl:128 * (tl + 1)],
                            wv_sb[:, k, 512:1024],
                            start=(k == 0), stop=(k == KD - 1))
                    dst = v_sb[:, t_abs, 512:1024]
                    if with_bv:
                        nc.vector.tensor_add(dst, ps[:], bv_bcast[:, 512:1024])
                    elif tl % 2 == 0:
                        nc.scalar.activation(dst, ps[:], AF.Identity)
                    else:
                        nc.vector.tensor_copy(dst, ps[:])

            def emit_out(h):
                st = live.pop(h)
                ost = otsg.tile([64, LV], BF, name="ot_st", tag="ot_st")
                for n in range(NCH):
                    obp = psB if n % 2 == 0 else psC
                    ob = obp.tile([64, 512], DT, name="ot_ps",
                                  tag="B" if n % 2 == 0 else "C")
                    nc.tensor.matmul(ob[:], st["up"][:],
                                     st["en"][:, 512 * n:512 * (n + 1)])
                    nc.vector.tensor_copy(ost[:, 512 * n:512 * (n + 1)], ob[:])
                    if h < 8 and n % 2 == 1:
                        emit_vg1(h, n // 2, n // 2 + 1)
                # one DMA per head keeps the sync queue free for transposes
                nc.sync.dma_start(ot_d[b, 64 * h:64 * (h + 1), :], ost[:])

            vg1_chunks = {}
            for h in range(H):
                if h < 8:
                    # reload xt chunk h for the deferred V columns
                    xtb = xtp.tile([128, KD, 512], BF, name="xt_v1", tag="xt")
                    nc.sync.dma_start(xtb[:], xsrc[:, :, 512 * h:512 * (h + 1)])
                    vg1_chunks[h] = xtb
                if h > 0:
                    emit_u(h - 1, 0, 16)
                emit_S(h, 0, 2)
                if h > 0:
                    emit_u(h - 1, 16, 32)
                emit_S(h, 2, 4)
                if h > 0:
                    emit_chain(h - 1)
                    emit_out(h - 1)
            emit_u(H - 1, 0, 16)
            emit_u(H - 1, 16, 32)
            emit_chain(H - 1)
            emit_out(H - 1)

            p3_scope.__exit__(None, None, None)
            # ---------- P4: final projection ----------
            p4_scope = nc.named_scope(f"p4_b{b}"); p4_scope.__enter__()
            osrc = ot_d[b].rearrange("(k p) n -> p k n", p=128)
            for c in range(NCH):
                oti = xtp.tile([128, KD, 512], BF, name="oti", tag="xt")
                nc.sync.dma_start(oti[:], osrc[:, :, 512 * c:512 * (c + 1)])
                for m in range(KD):
                    ps = psA.tile([128, 512], DT, name="f_ps", tag="A")
                    for k in range(KD):
                        nc.tensor.matmul(ps[:], wo_sb[:, k, 128 * m:128 * (m + 1)],
                                         oti[:, k, :],
                                         start=(k == 0), stop=(k == KD - 1))
                    st = fosg.tile([128, 512], DT, name="fin_st", tag="fin_st")
                    nc.scalar.activation(st[:], ps[:], AF.Identity,
                                         bias=bias_sb["bo"][:, m:m + 1])
                    nc.sync.dma_start(
                        out_d[b, 128 * m:128 * (m + 1), 512 * c:512 * (c + 1)],
                        st[:])
            p4_scope.__exit__(None, None, None)

    nc.compile()
    return nc


_nc_cache = {}


def _get_program(nb=NB, with_bv=False):
    key = (nb, with_bv)
    if key not in _nc_cache:
        _nc_cache[key] = build_program(nb, with_bv)
    return _nc_cache[key]


def make_in_maps(hidden_states, text_states, Wq, bq, Wk, bk, Wv, bv, Wo, bo):
    """Host-side staging: transpose to feature-major, shard batches."""
    import ml_dtypes
    f32 = np.float32
    bf16 = ml_dtypes.bfloat16
    hs = np.asarray(hidden_states, f32)
    ts = np.asarray(text_states, f32)
    xt_all = np.ascontiguousarray(hs.transpose(0, 2, 1)).astype(bf16)  # (B,D,LV)
    # Faithful to the reference's torch-style .view: text_states (LT, B, D)
    # reinterpreted in raw memory order as (B, LT, D), then feature-major.
    tt_all = np.ascontiguousarray(
        ts.reshape(B, LT, D).transpose(0, 2, 1)).astype(bf16)
    with_bv = bool(np.any(np.asarray(bv)))
    shared = {
        "wq": np.asarray(Wq, f32).astype(bf16),
        "wk": np.asarray(Wk, f32).astype(bf16),
        "wv": np.asarray(Wv, f32).astype(bf16),
        "wo": np.asarray(Wo, f32).astype(bf16),
        "bqs8": np.ascontiguousarray(np.asarray(bq, f32) * (SCALE * QK8)),
        "bk8": np.ascontiguousarray(np.asarray(bk, f32) * QK8),
        "bo": np.ascontiguousarray(np.asarray(bo, f32)),
    }
    if with_bv:
        shared["bv"] = np.ascontiguousarray(np.asarray(bv, f32))
    in_maps = []
    for c in range(N_CORES):
        sl = slice(c * NB, (c + 1) * NB)
        in_maps.append({
            "xt": np.ascontiguousarray(xt_all[sl]),
            "tt": np.ascontiguousarray(tt_all[sl]),
            **shared,
        })
    return in_maps, with_bv


def kernel(hidden_states, text_states, Wq, bq, Wk, bk, Wv, bv, Wo, bo):
    in_maps, with_bv = make_in_maps(hidden_states, text_states, Wq, bq,
                                    Wk, bk, Wv, bv, Wo, bo)
    nc = _get_program(with_bv=with_bv)
    res = bass_utils.run_bass_kernel_spmd(nc, in_maps, list(range(N_CORES)))
    out = np.empty((B, LV, D), np.float32)
    for c in range(N_CORES):
        o = res.results[c]["out"]                                  # (NB, D, LV)
        for j in range(NB):
            out[c * NB + j] = o[j].T
    return out



# revision 2
# speedup vs baseline: 1.1349x; 1.1349x over previous
"""CLIP cross-attention kernel for 8 TRN2 NeuronCores (v3).

Math (per batch b, head h):
  Q = (T @ Wq + bq) * scale           T = text_states[:, b, :]   (128, 1024)
  K = X @ Wk + bk                     X = hidden_states[b]       (4096, 1024)
  V = X @ Wv + bv
  S = Q_h @ K_h^T                     (128, 4096)
  E = exp(S); d = rowsum(E)
  out_h = E^T @ (E @ V_h) / d^2       (4096, 64)
  final = concat_h(out_h) @ Wo + bo

Sharding: batch across 8 cores (2 batches each), weights replicated.

v3 design (from the 1224us v2 baseline, PE busy 968us @ 78.7%):
 - K projection runs in fp8 DoubleRow (Wk scaled x32 on host, X as fp8):
   half the matmul instructions at 2 MACs/cycle. K only feeds attention
   weights, which tolerate fp8 noise (softmax-normalized, near-uniform
   attention averages it out). V / out-projection stay bf16: their noise
   passes straight through to the output.
 - Attention processed in HEAD PAIRS. Even/odd heads of a pair live at
   partitions 0:64 / 64:128 of qt/kt (k-slot = hp), so their S matmuls
   auto-derive tile_position=(0,0)/(64,0) -> run CONCURRENTLY in
   different row strips of the PE array. The out matmuls (M=64) are
   col-tiled into ONE [128,512] psum tile (tile_position (0,0)/(0,64))
   -> concurrent, and evacuate with a single DVE copy + one DMA per
   pair instead of per head.
 - V is computed in halves to halve its SBUF residency (32KB instead of
   64KB): heads 0-7 columns in P1, heads 8-15 columns re-computed into
   the same buffer mid-P3 (after pair 3's u-matmuls release it). The
   mid-P3 block doubles as dense PE filler while ACT drains exp work.
 - exp stays on ACT (only engine with activation); S-phase is ACT-paced
   at ~7us/pair, balanced against ~7us/pair of PE work.
 - 1/d^2 via per-partition scale AP on the u evacuation; biases ride
   activation evacuations.
 - All DMAs on nc.sync (ACT hwdge dispatch crashes the device).
"""
import sys
import numpy as np

sys.path.insert(0, '/opt/trn_rl_repo')

import concourse.bass as bass          # noqa: E402
import concourse.tile as tile          # noqa: E402
from concourse import bacc, mybir      # noqa: E402
from concourse import bass_utils       # noqa: E402
from contextlib import ExitStack       # noqa: E402

DT = mybir.dt.float32
BF = mybir.dt.bfloat16
F8 = mybir.dt.float8e4
AF = mybir.ActivationFunctionType
PM = mybir.MatmulPerfMode

B, LT, LV, D, H = 16, 128, 4096, 1024, 16
HD = D // H          # 64
NB = 2               # batches per core
N_CORES = 8
SCALE = HD ** -0.5
KD = D // 128        # 8
LVT = LV // 128      # 32
NCH = LV // 512      # 8
QK8 = 8.0            # fp8 storage scale for q/k
WKS = 32.0           # host premultiplier on Wk for fp8


def build_program(nb=NB, with_bv=False):
    nc = bacc.Bacc("TRN2", target_bir_lowering=False, debug=False)

    xt_d = nc.dram_tensor("xt", [nb, D, LV], BF, kind="ExternalInput")
    x8_d = nc.dram_tensor("x8", [nb, D, LV], F8, kind="ExternalInput")
    tt_d = nc.dram_tensor("tt", [nb, D, LT], BF, kind="ExternalInput")
    wk8_d = nc.dram_tensor("wk8", [D, D], F8, kind="ExternalInput")
    w_d = {nm: nc.dram_tensor(nm, [D, D], BF, kind="ExternalInput")
           for nm in ("wq", "wv", "wo")}
    b_d = {nm: nc.dram_tensor(nm, [D], DT, kind="ExternalInput")
           for nm in ("bqs8", "bk8", "bo")}
    if with_bv:
        b_d["bv"] = nc.dram_tensor("bv", [D], DT, kind="ExternalInput")
    out_d = nc.dram_tensor("out", [nb, D, LV], DT, kind="ExternalOutput")
    ot_d = nc.dram_tensor("ot_scratch", [nb, D, LV], BF)

    with tile.TileContext(nc) as tc, ExitStack() as top:
        ep = top.enter_context
        wk8p = ep(tc.tile_pool(name="wk8p", bufs=1))
        wvp = ep(tc.tile_pool(name="wvp", bufs=1))     # tags wvA/wvB
        wpool = ep(tc.tile_pool(name="wp", bufs=1))    # wq, wo cycle
        biasp = ep(tc.tile_pool(name="biasp", bufs=1))
        xtp = ep(tc.tile_pool(name="xtp", bufs=2))
        x8p = ep(tc.tile_pool(name="x8p", bufs=2))
        ktp = ep(tc.tile_pool(name="ktp", bufs=1))
        vp = ep(tc.tile_pool(name="vp", bufs=1))
        ttp = ep(tc.tile_pool(name="ttp", bufs=1))
        qtp = ep(tc.tile_pool(name="qtp", bufs=1))
        enp = ep(tc.tile_pool(name="enp", bufs=3))
        etp = ep(tc.tile_pool(name="etp", bufs=3))
        smp = ep(tc.tile_pool(name="smp", bufs=4))
        otsg = ep(tc.tile_pool(name="otsg", bufs=2))
        fosg = ep(tc.tile_pool(name="fosg", bufs=2))
        psA = ep(tc.tile_pool(name="psA", bufs=2, space="PSUM"))
        psB = ep(tc.tile_pool(name="psB", bufs=2, space="PSUM"))
        psC = ep(tc.tile_pool(name="psC", bufs=2, space="PSUM"))

        # Wk (fp8, x32) resident for the whole kernel
        wk8_sb = wk8p.tile([128, KD, D], F8, name="wk8", tag="wk8")
        wk8src = wk8_d.ap().rearrange("(k p) n -> p k n", p=128)
        for k in range(KD):
            nc.sync.dma_start(wk8_sb[:, k, :], wk8src[:, k, :])

        def load_w(nm, pool, tag, cols=None):
            c0, c1 = (0, D) if cols is None else cols
            t = pool.tile([128, KD, c1 - c0], BF, name=f"w_{tag}", tag=tag)
            src = w_d[nm].ap().rearrange("(k p) n -> p k n", p=128)
            for k in range(KD):
                nc.sync.dma_start(t[:, k, :], src[:, k, c0:c1])
            return t

        bias_sb = {}
        for nm in b_d:
            t = biasp.tile([128, KD], DT, name=f"b_{nm}", tag=f"b_{nm}")
            nc.sync.dma_start(t[:], b_d[nm].ap().rearrange("(k p) -> p k", p=128))
            bias_sb[nm] = t

        bv_bcast = None
        if with_bv:
            bv_row = biasp.tile([1, D], DT, tag="bv_row")
            nc.sync.dma_start(bv_row[:], b_d["bv"].ap().unsqueeze(0))
            ones_row = biasp.tile([1, 128], DT, tag="ones_row")
            nc.vector.memset(ones_row[:], 1.0)
            bv_bcast = biasp.tile([128, D], DT, tag="bv_bcast")
            for g in range(2):
                pb = psB.tile([128, 512], DT, name="bv_ps", tag="B")
                nc.tensor.matmul(pb[:], ones_row[:],
                                 bv_row[:, 512 * g:512 * (g + 1)])
                nc.vector.tensor_copy(bv_bcast[:, 512 * g:512 * (g + 1)], pb[:])

        for b in range(nb):
            # ---------- P1: K^T (fp8 DoubleRow) + V heads 0-7 (bf16) ----------
            p1_scope = nc.named_scope(f"p1_b{b}"); p1_scope.__enter__()
            wvA = load_w("wv", wvp, "wvA", (0, 512))
            wvB = load_w("wv", wvp, "wvB", (512, 1024))
            kt_sb = ktp.tile([128, KD, LV], F8, name="kt", tag="kt")
            v_sb = vp.tile([128, LVT, 512], BF, name="v", tag="v")
            xsrc = xt_d[b].rearrange("(k p) n -> p k n", p=128)
            x8src = x8_d[b].rearrange("(k p) n -> p k n", p=128)

            for c in range(NCH):
                x8t = x8p.tile([128, KD, 512], F8, name="x8c", tag="x8")
                nc.sync.dma_start(x8t[:], x8src[:, :, 512 * c:512 * (c + 1)])
                xtt = xtp.tile([128, KD, 512], BF, name="xtc", tag="xt")
                nc.sync.dma_start(xtt[:], xsrc[:, :, 512 * c:512 * (c + 1)])
                for m in range(KD):
                    ps = psA.tile([128, 512], DT, name="k_ps", tag="A")
                    for kp in range(KD // 2):
                        nc.tensor.matmul(
                            ps[:], wk8_sb[:, 2 * kp:2 * kp + 2, 128 * m:128 * (m + 1)],
                            x8t[:, 2 * kp:2 * kp + 2, :],
                            start=(kp == 0), stop=(kp == KD // 2 - 1),
                            perf_mode=PM.DoubleRow)
                    nc.scalar.activation(
                        kt_sb[:, m, 512 * c:512 * (c + 1)], ps[:], AF.Identity,
                        bias=bias_sb["bk8"][:, m:m + 1], scale=QK8 / WKS)
                for tl in range(4):
                    t_abs = 4 * c + tl
                    ps = psB.tile([128, 512], DT, name="v_ps", tag="B")
                    for k in range(KD):
                        nc.tensor.matmul(
                            ps[:], xtt[:, k, 128 * tl:128 * (tl + 1)], wvA[:, k, :],
                            start=(k == 0), stop=(k == KD - 1))
                    if with_bv:
                        nc.vector.tensor_add(v_sb[:, t_abs, :], ps[:],
                                             bv_bcast[:, 0:512])
                    else:
                        nc.vector.tensor_copy(v_sb[:, t_abs, :], ps[:])
            p1_scope.__exit__(None, None, None)

            # ---------- P2: Q^T (fp8) ----------
            p2_scope = nc.named_scope(f"p2_b{b}"); p2_scope.__enter__()
            tt_sb = ttp.tile([128, KD, LT], BF, name="tt", tag="tt")
            nc.sync.dma_start(tt_sb[:], tt_d[b].rearrange("(k p) t -> p k t", p=128))
            wq_sb = load_w("wq", wpool, "w")
            qt_sb = qtp.tile([128, KD, LT], F8, name="qt", tag="qt")
            for m in range(KD):
                ps = psC.tile([128, LT], DT, name="q_ps", tag="C")
                for k in range(KD):
                    nc.tensor.matmul(ps[:], wq_sb[:, k, 128 * m:128 * (m + 1)],
                                     tt_sb[:, k, :],
                                     start=(k == 0), stop=(k == KD - 1))
                nc.scalar.activation(qt_sb[:, m, :], ps[:], AF.Identity,
                                     bias=bias_sb["bqs8"][:, m:m + 1],
                                     scale=SCALE * QK8)
            wo_sb = load_w("wo", wpool, "w")   # prefetch during attention
            p2_scope.__exit__(None, None, None)
            p3_scope = nc.named_scope(f"p3_b{b}"); p3_scope.__enter__()

            # ---------- P3: attention, head-pair pipelined ----------
            live = {}

            def emit_S(hp, glo, ghi):
                if glo == 0:
                    for j in (0, 1):
                        live[(hp, j)] = {
                            "en": enp.tile([128, LV], BF, name=f"en{j}", tag="en"),
                            "et": etp.tile([128, LVT, 128], BF, name=f"et{j}",
                                           tag="et"),
                            "dparts": smp.tile([128, 4], DT, name=f"dp{j}",
                                               tag="dparts"),
                        }
                for g in range(glo, ghi):
                    pss = [psA.tile([128, 1024], DT, name="s_ps", tag="A")
                           for _ in (0, 1)]
                    for half in (0, 1):
                        c0 = 1024 * g + 512 * half
                        for j in (0, 1):
                            nc.tensor.matmul(
                                pss[j][:, 512 * half:512 * (half + 1)],
                                qt_sb[64 * j:64 * (j + 1), hp, :],
                                kt_sb[64 * j:64 * (j + 1), hp, c0:c0 + 512])
                    for j in (0, 1):
                        st = live[(hp, j)]
                        nc.scalar.activation(
                            st["en"][:, 1024 * g:1024 * (g + 1)], pss[j][:], AF.Exp,
                            scale=1.0 / (QK8 * QK8),
                            accum_out=st["dparts"][:, g:g + 1])
                        nc.sync.dma_start_transpose(
                            st["et"][:, 8 * g:8 * (g + 1), :],
                            st["en"][:, 1024 * g:1024 * (g + 1)])

            def emit_u(hp, tlo, thi):
                for j in (0, 1):
                    if tlo == 0:
                        live[(hp, j)]["ub"] = psC.tile([128, HD], DT,
                                                       name=f"u_ps{j}", tag="C")
                for t in range(tlo, thi):
                    for j in (0, 1):
                        st = live[(hp, j)]
                        c0 = HD * (2 * (hp % 4) + j)
                        nc.tensor.matmul(st["ub"][:], st["et"][:, t, :],
                                         v_sb[:, t, c0:c0 + HD],
                                         start=(t == 0), stop=(t == LVT - 1))

            def emit_chain(hp):
                for j in (0, 1):
                    st = live[(hp, j)]
                    dsum = smp.tile([128, 1], DT, name="dsum", tag=f"dsum{j}")
                    nc.vector.reduce_sum(dsum[:], st["dparts"][:],
                                         axis=mybir.AxisListType.X)
                    rd = smp.tile([128, 1], DT, name="rd", tag=f"rd{j}")
                    nc.vector.reciprocal(rd[:], dsum[:])
                    rr = smp.tile([128, 1], DT, name="rr", tag=f"rr{j}")
                    nc.vector.tensor_mul(rr[:], rd[:], rd[:])
                    up = smp.tile([128, HD], BF, name="up", tag=f"up{j}")
                    nc.scalar.activation(up[:], st["ub"][:], AF.Identity,
                                         scale=rr[:])
                    st["up"] = up

            def emit_out(hp):
                sts = [live.pop((hp, j)) for j in (0, 1)]
                ost = otsg.tile([128, LV], BF, name="ot_st", tag="ot_st")
                for n in range(NCH):
                    obp, tg = (psB, "B") if n % 2 == 0 else (psC, "C")
                    ob = obp.tile([128, 512], DT, name="ot_ps", tag=tg)
                    for j in (0, 1):
                        nc.tensor.matmul(ob[64 * j:64 * (j + 1), :],
                                         sts[j]["up"][:],
                                         sts[j]["en"][:, 512 * n:512 * (n + 1)],
                                         tile_position=(0, 64 * j))
                    nc.vector.tensor_copy(ost[:, 512 * n:512 * (n + 1)], ob[:])
                nc.sync.dma_start(ot_d[b, 128 * hp:128 * (hp + 1), :], ost[:])

            def emit_vfill(c):
                # V columns for heads 8-15, re-computed into v_sb after the
                # pair-0..3 u-matmuls release it; dense PE filler mid-P3.
                xtt = xtp.tile([128, KD, 512], BF, name="xt_vf", tag="xt")
                nc.sync.dma_start(xtt[:], xsrc[:, :, 512 * c:512 * (c + 1)])
                for tl in range(4):
                    t_abs = 4 * c + tl
                    ps = psB.tile([128, 512], DT, name="v_ps1", tag="B")
                    for k in range(KD):
                        nc.tensor.matmul(
                            ps[:], xtt[:, k, 128 * tl:128 * (tl + 1)], wvB[:, k, :],
                            start=(k == 0), stop=(k == KD - 1))
                    dst = v_sb[:, t_abs, :]
                    if with_bv:
                        nc.vector.tensor_add(dst, ps[:], bv_bcast[:, 512:1024])
                    elif tl % 2 == 0:
                        nc.scalar.activation(dst, ps[:], AF.Identity)
                    else:
                        nc.vector.tensor_copy(dst, ps[:])

            for hp in range(H // 2):
                if hp > 0:
                    emit_u(hp - 1, 0, 16)
                emit_S(hp, 0, 2)
                if hp > 0:
                    emit_u(hp - 1, 16, 32)
                emit_S(hp, 2, 4)
                if hp > 0:
                    emit_chain(hp - 1)
                    emit_out(hp - 1)
                if hp == 4:
                    for c in range(NCH):
                        emit_vfill(c)
            emit_u(H // 2 - 1, 0, 16)
            emit_u(H // 2 - 1, 16, 32)
            emit_chain(H // 2 - 1)
            emit_out(H // 2 - 1)
            p3_scope.__exit__(None, None, None)

            # ---------- P4: final projection ----------
            p4_scope = nc.named_scope(f"p4_b{b}"); p4_scope.__enter__()
            osrc = ot_d[b].rearrange("(k p) n -> p k n", p=128)
            for c in range(NCH):
                oti = xtp.tile([128, KD, 512], BF, name="oti", tag="xt")
                nc.sync.dma_start(oti[:], osrc[:, :, 512 * c:512 * (c + 1)])
                for m in range(KD):
                    ps = psA.tile([128, 512], DT, name="f_ps", tag="A")
                    for k in range(KD):
                        nc.tensor.matmul(ps[:], wo_sb[:, k, 128 * m:128 * (m + 1)],
                                         oti[:, k, :],
                                         start=(k == 0), stop=(k == KD - 1))
                    st = fosg.tile([128, 512], DT, name="fin_st", tag="fin_st")
                    nc.scalar.activation(st[:], ps[:], AF.Identity,
                                         bias=bias_sb["bo"][:, m:m + 1])
                    nc.sync.dma_start(
                        out_d[b, 128 * m:128 * (m + 1), 512 * c:512 * (c + 1)],
                        st[:])
            p4_scope.__exit__(None, None, None)

    nc.compile()
    return nc


_nc_cache = {}


def _get_program(nb=NB, with_bv=False):
    key = (nb, with_bv)
    if key not in _nc_cache:
        _nc_cache[key] = build_program(nb, with_bv)
    return _nc_cache[key]


def make_in_maps(hidden_states, text_states, Wq, bq, Wk, bk, Wv, bv, Wo, bo):
    """Host-side staging: transpose to feature-major, shard batches."""
    import ml_dtypes
    f32 = np.float32
    bf16 = ml_dtypes.bfloat16
    f8 = ml_dtypes.float8_e4m3
    hs = np.asarray(hidden_states, f32)
    ts = np.asarray(text_states, f32)
    xt_f32 = np.ascontiguousarray(hs.transpose(0, 2, 1))           # (B,D,LV)
    xt_all = xt_f32.astype(bf16)
    x8_all = xt_f32.astype(f8)
    # Faithful to the reference's torch-style .view: text_states (LT, B, D)
    # reinterpreted in raw memory order as (B, LT, D), then feature-major.
    tt_all = np.ascontiguousarray(
        ts.reshape(B, LT, D).transpose(0, 2, 1)).astype(bf16)
    with_bv = bool(np.any(np.asarray(bv)))
    shared = {
        "wk8": (np.asarray(Wk, f32) * WKS).astype(f8),
        "wq": np.asarray(Wq, f32).astype(bf16),
        "wv": np.asarray(Wv, f32).astype(bf16),
        "wo": np.asarray(Wo, f32).astype(bf16),
        "bqs8": np.ascontiguousarray(np.asarray(bq, f32) * (SCALE * QK8)),
        "bk8": np.ascontiguousarray(np.asarray(bk, f32) * QK8),
        "bo": np.ascontiguousarray(np.asarray(bo, f32)),
    }
    if with_bv:
        shared["bv"] = np.ascontiguousarray(np.asarray(bv, f32))
    in_maps = []
    for c in range(N_CORES):
        sl = slice(c * NB, (c + 1) * NB)
        in_maps.append({
            "xt": np.ascontiguousarray(xt_all[sl]),
            "x8": np.ascontiguousarray(x8_all[sl]),
            "tt": np.ascontiguousarray(tt_all[sl]),
            **shared,
        })
    return in_maps, with_bv


def kernel(hidden_states, text_states, Wq, bq, Wk, bk, Wv, bv, Wo, bo):
    in_maps, with_bv = make_in_maps(hidden_states, text_states, Wq, bq,
                                    Wk, bk, Wv, bv, Wo, bo)
    nc = _get_program(with_bv=with_bv)
    res = bass_utils.run_bass_kernel_spmd(nc, in_maps, list(range(N_CORES)))
    out = np.empty((B, LV, D), np.float32)
    for c in range(N_CORES):
        o = res.results[c]["out"]                                  # (NB, D, LV)
        for j in range(NB):
            out[c * NB + j] = o[j].T
    return out
